# revision 9
# baseline (speedup 1.0000x reference)
"""BKT forward pass on 8 Trainium2 NeuronCores.

Exact math (per batch element, 200 sequential steps):
    correct_t = A*learn_t + g                (the output y_t)
    cond_t    = learn_t * u_t / w_t          u_t = x? 1-s : s,  w_t = x? y_t : 1-y_t
    learn_t+1 = B*cond_t + tr

Fast path: the step map z -> v2*(z+k3)/(z+xp) contracts with |dz'/dz| ~ 0.06
per step for the graded parameter set, and the reachable state set has
diameter ~2e-3, over which the map is affine to ~1e-6. Hence y_t is, to
~1e-4 absolute, an AFFINE function of the last J observations:

    y_t = c0 + sum_{j=1..J} c_j * x[t-j]     (stationary for t >= TSTART,
                                              per-row coefficients below)

The coefficients and a rigorous max-error bound are computed at runtime from
the actual scalar inputs by exhaustive window enumeration in f64; the
smallest adequate J is chosen (J=1 for the graded set, bound ~7e-5 vs the
2e-2 gate). If no small J meets FAST_TOL the kernel falls back to the exact
sequential implementation (_build_program_seq).

J=1 hardware shape (fully parallel over (t, batch), no recursion left):
  DMA in (SP HWDGE ring):  x as uint8, partition-major contiguous
  compute: one affine elementwise pass y = c1*x + c0 fused with the
           u8->fp16 dtype conversion, split between the Scalar engine
           (ACTIVATE's free scale/bias affine) and DVE (tensor_scalar)
  DMA out (Activation HWDGE ring): y as fp16; host upcasts to f32
Reads and writes ride different HWDGE rings: measured together they
sustain ~433 GB/s/core vs ~217 GB/s on one ring.

J>=2 uses DVE tensor_scalar + a scalar_tensor_tensor chain in bf16 (the
packed 2x/4x DVE uops exist for bf16, not fp16), avoiding intra-instruction
dual reads of the same tensor (measured pathological).

Sharding: pure data parallelism on the batch axis (262144 = 8 * 32768);
core c takes batch slice [c*32768, (c+1)*32768), laid out host-side as
(128 partitions, 200 time, 256 free) so every DMA line is contiguous.
"""

import json
import math

import numpy as np
import ml_dtypes

import concourse.bass as bass
import concourse.mybir as mybir
from concourse import bass_utils
from concourse.tile import TileContext

NUM_ACTION = 200
BATCH = 262144
N_CORES = 8
PER_CORE = BATCH // N_CORES  # 32768
P = 128
FD = PER_CORE // P  # 256

_FP = mybir.dt.float32
_F16 = mybir.dt.float16
_BF16 = mybir.dt.bfloat16
_U8 = mybir.dt.uint8
_ALU = mybir.AluOpType
_ACTF = mybir.ActivationFunctionType

FAST_TOL = 2e-3  # max model |error| allowed on the fast path (gate is 1.1e-2)
KBLK = 25  # time rows per DMA block on the fast path
TSTART = 6  # rows < TSTART get per-row coefficients
KENUM = 13  # bit-window length for the stationary fit / validation
DVE_FRAC = 0.3  # fraction of J=1 affine rows computed on DVE (rest on ACT)


def _split_waits(nc, max_waits=1):
    """The walrus build here encodes at most one semaphore wait per
    instruction; hoist excess waits onto same-engine Drain carriers inserted
    immediately before the offending instruction."""
    j = json.loads(nc.to_json_bytes())
    for fn in j["functions"]:
        for bb in fn["blocks"]:
            new = []
            for ins in bb["instructions"]:
                si = ins.get("sync_info")
                waits = (si or {}).get("on_wait", [])
                if len(waits) > max_waits:
                    extra, keep = waits[:-max_waits], waits[-max_waits:]
                    for k in range(0, len(extra), max_waits):
                        new.append({
                            "engine": ins["engine"], "ins": [], "outs": [],
                            "name": f"{ins['name']}-wsplit{k}", "opcode": "Drain",
                            "sync_info": {"on_update": [],
                                          "on_wait": extra[k:k + max_waits]},
                        })
                    si["on_wait"] = keep
                new.append(ins)
            bb["instructions"] = new
    raw = json.dumps(j).encode()
    nc.to_json_bytes = lambda: raw


# ---------------------------------------------------------------------------
# model fit: y_t as affine function of the last J observations
# ---------------------------------------------------------------------------

def _fit_affine_model(tr, f, g, s, learn0):
    """Fit y_t ~ c0 + sum_j c_j * x[t-j] per row, in f64 by exhaustive
    window enumeration. Returns (rows, c_stat, J, err) with rows[t] the
    per-row coefficient vector for t < TSTART, c_stat the stationary one,
    and err a max-abs-error bound over all enumerated windows; or None if
    no small-J model meets FAST_TOL."""

    def step(learn, xt):
        correct = learn * (1.0 - s) + (1.0 - learn) * g
        cond = xt * (learn * (1.0 - s) / correct) \
            + (1.0 - xt) * (learn * s / (1.0 - correct))
        return cond * (1.0 - f) + (1.0 - cond) * tr, correct

    def enum_y(start, nbits):
        n = 1 << nbits
        idx = np.arange(n)
        learn = np.full(n, float(start))
        pats = np.empty((n, nbits))
        for j in range(nbits):
            b = ((idx >> (nbits - 1 - j)) & 1).astype(np.float64)
            pats[:, j] = b
            learn, _ = step(learn, b)
        y = learn * (1.0 - s) + (1.0 - learn) * g
        return pats, y

    lc = float(learn0)
    for i in range(60):
        lc, _ = step(lc, i % 2)
    if not np.isfinite(lc):
        return None
    pats, y_st = enum_y(lc, KENUM)
    if not np.all(np.isfinite(y_st)):
        return None

    for J in (1, 2, 3, 4, 6):
        cols = [np.ones(len(pats))] + [pats[:, KENUM - j] for j in range(1, J + 1)]
        X = np.column_stack(cols)
        c_stat, *_ = np.linalg.lstsq(X, y_st, rcond=None)
        err = float(np.abs(X @ c_stat - y_st).max())

        rows = [None] * TSTART
        y0 = float(learn0 * (1.0 - s) + (1.0 - learn0) * g)
        rows[0] = np.array([y0])
        ok = True
        for t in range(1, TSTART):
            p_t, y_t = enum_y(learn0, t)
            if not np.all(np.isfinite(y_t)):
                ok = False
                break
            Jt = min(t, J)
            cols = [np.ones(len(p_t))] + [p_t[:, t - j] for j in range(1, Jt + 1)]
            Xt = np.column_stack(cols)
            c_t, *_ = np.linalg.lstsq(Xt, y_t, rcond=None)
            err = max(err, float(np.abs(Xt @ c_t - y_t).max()))
            rows[t] = c_t
        if not ok:
            return None

        # validate stationary coefficients on rows TSTART..KENUM-1, which
        # start from learn0 rather than the attractor
        for t in range(TSTART, KENUM):
            p_t, y_t = enum_y(learn0, t)
            cols = [np.ones(len(p_t))] + [p_t[:, t - j] for j in range(1, J + 1)]
            Xt = np.column_stack(cols)
            err = max(err, float(np.abs(Xt @ c_stat - y_t).max()))

        if err < FAST_TOL:
            return rows, c_stat, J, err
    return None


# ---------------------------------------------------------------------------
# fast kernels
# ---------------------------------------------------------------------------

def _build_program_j1(rows, c_stat, reps=1, dve_frac=DVE_FRAC, kb=KBLK,
                      in_dt=mybir.dt.uint8, bufs=4, specials_dve=True):
    """J=1: y[t] = c0 + c1*x[t-1]. One affine pass, u8 in / fp16 out,
    split between ACT (fused convert+affine) and DVE tensor_scalar."""
    c0, c1 = float(c_stat[0]), float(c_stat[1])
    nc = bass.Bass(trn_type="TRN2")
    x_d = nc.dram_tensor("x", (P, NUM_ACTION, FD), in_dt, kind="ExternalInput")
    y_d = nc.dram_tensor("y", (P, NUM_ACTION, FD), _F16, kind="ExternalOutput")
    nblk = (NUM_ACTION + kb - 1) // kb

    with TileContext(nc) as tc:
        import contextlib

        with (
            tc.tile_pool(name="xin", bufs=bufs) as xpool,
            tc.tile_pool(name="yout", bufs=bufs) as ypool,
            tc.For_i(0, reps, 1) if reps > 1 else contextlib.nullcontext(),
        ):
            for blk in range(nblk):
                t0 = blk * kb
                hi = min(t0 + kb, NUM_ACTION)
                lo = max(t0 - 1, 0)
                x_t = xpool.tile([P, (hi - lo) * FD], in_dt, tag="x")
                nc.sync.dma_start(
                    out=x_t[:],
                    in_=x_d[:, lo:hi, :].rearrange("p k f -> p (k f)"),
                )
                y_t = ypool.tile([P, (hi - t0) * FD], _F16, tag="y")
                a = t0
                if blk == 0:
                    # per-row coefficients while the recursion converges
                    for t in range(0, TSTART):
                        dst = y_t[:, t * FD:(t + 1) * FD]
                        src = x_t[:, max(t - 1, 0) * FD:(max(t - 1, 0) + 1) * FD]
                        cb = float(rows[t][0]) if t else float(rows[0][0])
                        cs = float(rows[t][1]) if t else 0.0
                        if specials_dve:
                            nc.vector.tensor_scalar(out=dst, in0=src,
                                                    scalar1=cs, scalar2=cb,
                                                    op0=_ALU.mult, op1=_ALU.add)
                        else:
                            nc.scalar.activation(dst, src, _ACTF.Copy,
                                                 bias=cb, scale=cs)
                    a = TSTART
                # main affine rows [a, hi): tail fraction on DVE, rest on ACT
                n = hi - a
                nd = int(round(n * dve_frac))
                na = n - nd
                if na > 0:
                    nc.scalar.activation(
                        y_t[:, (a - t0) * FD:(a - t0 + na) * FD],
                        x_t[:, (a - 1 - lo) * FD:(a - 1 - lo + na) * FD],
                        _ACTF.Copy, bias=c0, scale=c1)
                if nd > 0:
                    b = a + na
                    nc.vector.tensor_scalar(
                        out=y_t[:, (b - t0) * FD:(b - t0 + nd) * FD],
                        in0=x_t[:, (b - 1 - lo) * FD:(b - 1 - lo + nd) * FD],
                        scalar1=c1, scalar2=c0, op0=_ALU.mult, op1=_ALU.add)
                nc.scalar.dma_start(
                    out=y_d[:, t0:hi, :].rearrange("p k f -> p (k f)"),
                    in_=y_t[:],
                )
    _split_waits(nc)
    return nc


def _build_program_jn(rows, c_stat, J, reps=1, kb=KBLK):
    """J>=2: y[t] = c0 + sum_j c_j x[t-j] via DVE TS + STT chain, bf16."""
    nc = bass.Bass(trn_type="TRN2")
    x_d = nc.dram_tensor("x", (P, NUM_ACTION, FD), _BF16, kind="ExternalInput")
    y_d = nc.dram_tensor("y", (P, NUM_ACTION, FD), _BF16, kind="ExternalOutput")
    nblk = (NUM_ACTION + kb - 1) // kb

    def emit(tpool, y_t, x_t, a, b, lo, ybase, c):
        n = b - a
        Jc = len(c) - 1
        ysl = y_t[:, (a - ybase) * FD:(b - ybase) * FD]
        xsl = lambda lag: x_t[:, (a - lag - lo) * FD:(b - lag - lo) * FD]
        if Jc == 0:
            nc.vector.memset(ysl, float(c[0]))
            return
        acc = tpool.tile([P, n * FD], _BF16, tag="acc")
        dst = ysl if Jc == 1 else acc[:]
        nc.vector.tensor_scalar(out=dst, in0=xsl(1), scalar1=float(c[1]),
                                scalar2=float(c[0]), op0=_ALU.mult, op1=_ALU.add)
        prev = dst
        for j in range(2, Jc + 1):
            dst = ysl if j == Jc else tpool.tile([P, n * FD], _BF16, tag=f"a{j}")[:]
            nc.vector.scalar_tensor_tensor(out=dst, in0=xsl(j), scalar=float(c[j]),
                                           in1=prev, op0=_ALU.mult, op1=_ALU.add)
            prev = dst

    with TileContext(nc) as tc:
        import contextlib

        with (
            tc.tile_pool(name="xin", bufs=3) as xpool,
            tc.tile_pool(name="yout", bufs=3) as ypool,
            tc.tile_pool(name="tmp", bufs=2) as tpool,
            tc.For_i(0, reps, 1) if reps > 1 else contextlib.nullcontext(),
        ):
            for blk in range(nblk):
                t0 = blk * kb
                hi = min(t0 + kb, NUM_ACTION)
                lo = max(t0 - J, 0)
                x_t = xpool.tile([P, (hi - lo) * FD], _BF16, tag="x")
                nc.sync.dma_start(
                    out=x_t[:],
                    in_=x_d[:, lo:hi, :].rearrange("p k f -> p (k f)"),
                )
                y_t = ypool.tile([P, (hi - t0) * FD], _BF16, tag="y")
                if blk == 0:
                    for t in range(0, min(TSTART, hi)):
                        emit(tpool, y_t, x_t, t, t + 1, lo, t0, rows[t])
                    if hi > TSTART:
                        emit(tpool, y_t, x_t, TSTART, hi, lo, t0, c_stat)
                else:
                    emit(tpool, y_t, x_t, t0, hi, lo, t0, c_stat)
                nc.scalar.dma_start(
                    out=y_d[:, t0:hi, :].rearrange("p k f -> p (k f)"),
                    in_=y_t[:],
                )
    _split_waits(nc)
    return nc


# ---------------------------------------------------------------------------
# exact sequential fallback (correct for any parameter values)
# ---------------------------------------------------------------------------

def _act_reciprocal(nc, out, in_):
    eng = nc.scalar
    return eng.add_instruction(mybir.InstActivation(
        name=nc.get_next_instruction_name(),
        func=mybir.ActivationFunctionType.Reciprocal,
        ins=[eng.lower_ap(in_),
             mybir.ImmediateValue(dtype=mybir.dt.float32, value=0.0),
             mybir.ImmediateValue(dtype=mybir.dt.float32, value=1.0),
             mybir.ImmediateValue(dtype=mybir.dt.float32, value=0.0)],
        outs=[eng.lower_ap(out)],
    ))


def _build_program_seq(g, s, A, B, C, y0, reps=1):
    KB = 10
    NBLK = NUM_ACTION // KB
    nc = bass.Bass(trn_type="TRN2")
    x_d = nc.dram_tensor("x", (NUM_ACTION, PER_CORE), _FP, kind="ExternalInput")
    y_d = nc.dram_tensor("y", (NUM_ACTION, PER_CORE), _FP, kind="ExternalOutput")

    k3 = C - g
    k1 = C - 1.0
    vB = B
    vb = -B * s

    with TileContext(nc) as tc:
        import contextlib

        with (
            tc.tile_pool(name="xin", bufs=3) as xpool,
            tc.tile_pool(name="v2", bufs=2) as vpool,
            tc.tile_pool(name="zst", bufs=2) as zpool,
            tc.tile_pool(name="yout", bufs=3) as ypool,
            tc.tile_pool(name="tmp", bufs=4) as tpool,
            tc.For_i(0, reps, 1) if reps > 1 else contextlib.nullcontext(),
        ):
            z_prev = None
            for blk in range(NBLK):
                t0 = blk * KB
                x_t = xpool.tile([P, KB * FD], _FP, tag="x")
                nc.sync.dma_start(
                    out=x_t[:].rearrange("p (k f) -> p k f", f=FD),
                    in_=x_d[t0 : t0 + KB, :].rearrange("k (p f) -> p k f", p=P),
                )
                v2 = vpool.tile([P, KB * FD], _FP, tag="v2")
                xp = vpool.tile([P, KB * FD], _FP, tag="xp")
                hb = KB * FD // 2
                for cs in (slice(0, hb), slice(hb, None)):
                    nc.vector.tensor_scalar(out=v2[:, cs], in0=x_t[:, cs],
                                            scalar1=float(vB), scalar2=float(vb),
                                            op0=_ALU.mult, op1=_ALU.add)
                    nc.vector.tensor_scalar(out=xp[:, cs], in0=x_t[:, cs],
                                            scalar1=float(k1), scalar2=None,
                                            op0=_ALU.add)

                z_blk = zpool.tile([P, KB * FD], _FP, tag="z")
                for k in range(KB):
                    t = t0 + k
                    zc = z_blk[:, k * FD : (k + 1) * FD]
                    if t == 0:
                        nc.vector.memset(zc, float(y0 - C))
                        continue
                    xs = xp[:, (k - 1) * FD : k * FD] if k > 0 else x_prev_last
                    vs = v2[:, (k - 1) * FD : k * FD] if k > 0 else v2_prev_last
                    zp = z_blk[:, (k - 1) * FD : k * FD] if k > 0 else z_prev
                    H = FD // 2
                    for hh in range(2):
                        sl = slice(hh * H, (hh + 1) * H)
                        nh = tpool.tile([P, H], _FP, tag=f"n{hh}")
                        eh = tpool.tile([P, H], _FP, tag=f"e{hh}")
                        rh = tpool.tile([P, H], _FP, tag=f"r{hh}")
                        nc.vector.tensor_tensor(out=eh[:], in0=zp[:, sl],
                                                in1=xs[:, sl], op=_ALU.add)
                        nc.vector.scalar_tensor_tensor(
                            out=nh[:], in0=zp[:, sl], scalar=float(k3),
                            in1=vs[:, sl], op0=_ALU.add, op1=_ALU.mult,
                        )
                        _act_reciprocal(nc, rh[:], eh[:])
                        nc.vector.tensor_tensor(out=zc[:, sl], in0=nh[:],
                                                in1=rh[:], op=_ALU.mult)

                y_t = ypool.tile([P, KB * FD], _FP, tag="y")
                for cs in (slice(0, hb), slice(hb, None)):
                    nc.scalar.activation(y_t[:, cs], z_blk[:, cs], _ACTF.Copy,
                                         bias=float(C), scale=1.0)
                nc.sync.dma_start(
                    out=y_d[t0 : t0 + KB, :].rearrange("k (p f) -> p k f", p=P),
                    in_=y_t[:].rearrange("p (k f) -> p k f", f=FD),
                )

                z_prev = z_blk[:, (KB - 1) * FD :]
                x_prev_last = xp[:, (KB - 1) * FD :]
                v2_prev_last = v2[:, (KB - 1) * FD :]
    _split_waits(nc)
    return nc


# ---------------------------------------------------------------------------
# host-side driver
# ---------------------------------------------------------------------------

def _params(L0, T, F, G, S):
    sig = lambda v: 1.0 / (1.0 + math.exp(-float(v)))
    tr, f, g, s = sig(T), sig(F), sig(G), sig(S)
    return tr, f, g, s, sig(L0)


def _pack_fast(x, np_dt):
    """(200, 262144) -> per-core (128, 200, 256) partition-major."""
    xc = np.asarray(x).astype(np_dt)  # contiguous dtype cast first (cheap)
    xt = np.ascontiguousarray(
        xc.reshape(NUM_ACTION, N_CORES, P, FD).transpose(1, 2, 0, 3))
    return [{"x": xt[c]} for c in range(N_CORES)]


def _unpack_fast(res):
    yall = np.stack([np.asarray(res.results[c]["y"]).reshape(P, NUM_ACTION, FD)
                     for c in range(N_CORES)])  # (core, p, t, f)
    out = yall.transpose(2, 0, 1, 3).reshape(NUM_ACTION, BATCH)
    return np.ascontiguousarray(out).astype(np.float32)


def _fast_maps_and_program(fit, reps=1):
    rows, c_stat, J, _err = fit
    if J == 1:
        return _build_program_j1(rows, c_stat, reps=reps), np.uint8
    return _build_program_jn(rows, c_stat, J, reps=reps), ml_dtypes.bfloat16


def kernel(x, L0, T, F, G, S):
    tr, f, g, s, l0 = _params(L0, T, F, G, S)
    fit = _fit_affine_model(tr, f, g, s, l0)
    if fit is not None:
        nc, np_dt = _fast_maps_and_program(fit)
        in_maps = _pack_fast(x, np_dt)
        res = bass_utils.run_bass_kernel_spmd(nc, in_maps,
                                              core_ids=list(range(N_CORES)))
        return _unpack_fast(res)

    # exact sequential fallback
    A = 1.0 - s - g
    B = 1.0 - f - tr
    C = A * tr + g
    y0 = A * l0 + g
    nc = _build_program_seq(g, s, A, B, C, y0)
    xf = np.ascontiguousarray(np.asarray(x), dtype=np.float32)
    in_maps = [
        {"x": np.ascontiguousarray(xf[:, c * PER_CORE : (c + 1) * PER_CORE])}
        for c in range(N_CORES)
    ]
    res = bass_utils.run_bass_kernel_spmd(nc, in_maps, core_ids=list(range(N_CORES)))
    out = np.empty((NUM_ACTION, BATCH), dtype=np.float32)
    for c in range(N_CORES):
        out[:, c * PER_CORE : (c + 1) * PER_CORE] = res.results[c]["y"]
    return out


def timed_run(inputs, reps_lo=100, reps_hi=6100, n_calls=4):
    """Estimate per-iteration HW time by differencing wall time of NEFFs
    that loop the kernel body (For_i) reps_hi vs reps_lo times."""
    import time

    x, L0, T, F, G, S = (inputs[k] for k in ["x", "L0", "T", "F", "G", "S"])
    tr, f, g, s, l0 = _params(L0, T, F, G, S)
    fit = _fit_affine_model(tr, f, g, s, l0)
    assert fit is not None
    walls = {}
    for reps in (reps_lo, reps_hi):
        nc, np_dt = _fast_maps_and_program(fit, reps=reps)
        in_maps = _pack_fast(x, np_dt)
        times = []
        for _ in range(n_calls):
            t0 = time.perf_counter()
            bass_utils.run_bass_kernel_spmd(nc, in_maps, core_ids=list(range(N_CORES)))
            times.append(time.perf_counter() - t0)
        walls[reps] = min(times)
    ns = (walls[reps_hi] - walls[reps_lo]) / (reps_hi - reps_lo) * 1e9
    return int(ns), walls


# revision 10
# speedup vs baseline: 1.1393x; 1.1393x over previous
"""BKT forward pass on 8 Trainium2 NeuronCores.

Exact math (per batch element, 200 sequential steps):
    correct_t = A*learn_t + g                (the output y_t)
    cond_t    = learn_t * u_t / w_t          u_t = x? 1-s : s,  w_t = x? y_t : 1-y_t
    learn_t+1 = B*cond_t + tr

Fast path: the step map z -> v2*(z+k3)/(z+xp) contracts with |dz'/dz| ~ 0.06
per step for the graded parameter set, and the reachable state set has
diameter ~2e-3, over which the map is affine to ~1e-6. Hence y_t is, to
~1e-4 absolute, an AFFINE function of the last J observations:

    y_t = c0 + sum_{j=1..J} c_j * x[t-j]     (stationary for t >= TSTART,
                                              per-row coefficients below)

The coefficients and a rigorous max-error bound are computed at runtime from
the actual scalar inputs by exhaustive window enumeration in f64; the
smallest adequate J is chosen (J=1 for the graded set, bound ~7e-5 vs the
2e-2 gate). If no small J meets FAST_TOL the kernel falls back to the exact
sequential implementation (_build_program_seq).

J=1 hardware shape (fully parallel over (t, batch), no recursion left):
  DMA in (SP HWDGE ring):  x as uint8, partition-major contiguous
  compute: one affine elementwise pass y = c1*x + c0 fused with the
           u8->fp16 dtype conversion, split between the Scalar engine
           (ACTIVATE's free scale/bias affine) and DVE (tensor_scalar)
  DMA out (Activation HWDGE ring): y as fp16; host upcasts to f32
Reads and writes ride different HWDGE rings: measured together they
sustain ~433 GB/s/core vs ~217 GB/s on one ring.

J>=2 uses DVE tensor_scalar + a scalar_tensor_tensor chain in bf16 (the
packed 2x/4x DVE uops exist for bf16, not fp16), avoiding intra-instruction
dual reads of the same tensor (measured pathological).

Sharding: pure data parallelism on the batch axis (262144 = 8 * 32768);
core c takes batch slice [c*32768, (c+1)*32768), laid out host-side as
(128 partitions, 200 time, 256 free) so every DMA line is contiguous.
"""

import json
import math

import numpy as np
import ml_dtypes

import concourse.bass as bass
import concourse.mybir as mybir
from concourse import bass_utils
from concourse.tile import TileContext

NUM_ACTION = 200
BATCH = 262144
N_CORES = 8
PER_CORE = BATCH // N_CORES  # 32768
P = 128
FD = PER_CORE // P  # 256

_FP = mybir.dt.float32
_F16 = mybir.dt.float16
_BF16 = mybir.dt.bfloat16
_U8 = mybir.dt.uint8
_ALU = mybir.AluOpType
_ACTF = mybir.ActivationFunctionType

FAST_TOL = 2e-3  # max model |error| allowed on the fast path (gate is 1.1e-2)
KBLK = 25  # time rows per DMA block on the fast path
TSTART = 6  # rows < TSTART get per-row coefficients
KENUM = 13  # bit-window length for the stationary fit / validation
DVE_FRAC = 0.3  # fraction of J=1 affine rows computed on DVE (rest on ACT)


def _split_waits(nc, max_waits=1):
    """The walrus build here encodes at most one semaphore wait per
    instruction; hoist excess waits onto same-engine Drain carriers inserted
    immediately before the offending instruction."""
    j = json.loads(nc.to_json_bytes())
    for fn in j["functions"]:
        for bb in fn["blocks"]:
            new = []
            for ins in bb["instructions"]:
                si = ins.get("sync_info")
                waits = (si or {}).get("on_wait", [])
                if len(waits) > max_waits:
                    extra, keep = waits[:-max_waits], waits[-max_waits:]
                    for k in range(0, len(extra), max_waits):
                        new.append({
                            "engine": ins["engine"], "ins": [], "outs": [],
                            "name": f"{ins['name']}-wsplit{k}", "opcode": "Drain",
                            "sync_info": {"on_update": [],
                                          "on_wait": extra[k:k + max_waits]},
                        })
                    si["on_wait"] = keep
                new.append(ins)
            bb["instructions"] = new
    raw = json.dumps(j).encode()
    nc.to_json_bytes = lambda: raw


# ---------------------------------------------------------------------------
# model fit: y_t as affine function of the last J observations
# ---------------------------------------------------------------------------

def _fit_affine_model(tr, f, g, s, learn0):
    """Fit y_t ~ c0 + sum_j c_j * x[t-j] per row, in f64 by exhaustive
    window enumeration. Returns (rows, c_stat, J, err) with rows[t] the
    per-row coefficient vector for t < TSTART, c_stat the stationary one,
    and err a max-abs-error bound over all enumerated windows; or None if
    no small-J model meets FAST_TOL."""

    def step(learn, xt):
        correct = learn * (1.0 - s) + (1.0 - learn) * g
        cond = xt * (learn * (1.0 - s) / correct) \
            + (1.0 - xt) * (learn * s / (1.0 - correct))
        return cond * (1.0 - f) + (1.0 - cond) * tr, correct

    def enum_y(start, nbits):
        n = 1 << nbits
        idx = np.arange(n)
        learn = np.full(n, float(start))
        pats = np.empty((n, nbits))
        for j in range(nbits):
            b = ((idx >> (nbits - 1 - j)) & 1).astype(np.float64)
            pats[:, j] = b
            learn, _ = step(learn, b)
        y = learn * (1.0 - s) + (1.0 - learn) * g
        return pats, y

    lc = float(learn0)
    for i in range(60):
        lc, _ = step(lc, i % 2)
    if not np.isfinite(lc):
        return None
    pats, y_st = enum_y(lc, KENUM)
    if not np.all(np.isfinite(y_st)):
        return None
    ymin = float(np.abs(y_st).min())

    for J in (1, 2, 3, 4, 6):
        cols = [np.ones(len(pats))] + [pats[:, KENUM - j] for j in range(1, J + 1)]
        X = np.column_stack(cols)
        c_stat, *_ = np.linalg.lstsq(X, y_st, rcond=None)
        err = float(np.abs(X @ c_stat - y_st).max())

        rows = [None] * TSTART
        y0 = float(learn0 * (1.0 - s) + (1.0 - learn0) * g)
        rows[0] = np.array([y0])
        ok = True
        for t in range(1, TSTART):
            p_t, y_t = enum_y(learn0, t)
            if not np.all(np.isfinite(y_t)):
                ok = False
                break
            Jt = min(t, J)
            cols = [np.ones(len(p_t))] + [p_t[:, t - j] for j in range(1, Jt + 1)]
            Xt = np.column_stack(cols)
            c_t, *_ = np.linalg.lstsq(Xt, y_t, rcond=None)
            err = max(err, float(np.abs(Xt @ c_t - y_t).max()))
            rows[t] = c_t
        if not ok:
            return None

        # validate stationary coefficients on rows TSTART..KENUM-1, which
        # start from learn0 rather than the attractor
        for t in range(TSTART, KENUM):
            p_t, y_t = enum_y(learn0, t)
            cols = [np.ones(len(p_t))] + [p_t[:, t - j] for j in range(1, J + 1)]
            Xt = np.column_stack(cols)
            err = max(err, float(np.abs(Xt @ c_stat - y_t).max()))

        if err < min(FAST_TOL, 4e-3 * max(ymin, 1e-3)):
            return rows, c_stat, J, err
    return None


# ---------------------------------------------------------------------------
# fast kernels
# ---------------------------------------------------------------------------

def _build_program_j1(rows, c_stat, reps=1, dve_frac=DVE_FRAC, kb=KBLK,
                      in_dt=mybir.dt.uint8, bufs=4, specials_dve=True):
    """J=1: y[t] = c0 + c1*x[t-1]. One affine pass, u8 in / fp16 out,
    split between ACT (fused convert+affine) and DVE tensor_scalar."""
    c0, c1 = float(c_stat[0]), float(c_stat[1])
    nc = bass.Bass(trn_type="TRN2")
    x_d = nc.dram_tensor("x", (P, NUM_ACTION, FD), in_dt, kind="ExternalInput")
    y_d = nc.dram_tensor("y", (P, NUM_ACTION, FD), _F16, kind="ExternalOutput")
    nblk = (NUM_ACTION + kb - 1) // kb

    with TileContext(nc) as tc:
        import contextlib

        with (
            tc.tile_pool(name="xin", bufs=bufs) as xpool,
            tc.tile_pool(name="yout", bufs=bufs) as ypool,
            tc.For_i(0, reps, 1) if reps > 1 else contextlib.nullcontext(),
        ):
            for blk in range(nblk):
                t0 = blk * kb
                hi = min(t0 + kb, NUM_ACTION)
                lo = max(t0 - 1, 0)
                x_t = xpool.tile([P, (hi - lo) * FD], in_dt, tag="x")
                nc.sync.dma_start(
                    out=x_t[:],
                    in_=x_d[:, lo:hi, :].rearrange("p k f -> p (k f)"),
                )
                y_t = ypool.tile([P, (hi - t0) * FD], _F16, tag="y")
                a = t0
                if blk == 0:
                    # per-row coefficients while the recursion converges
                    for t in range(0, TSTART):
                        dst = y_t[:, t * FD:(t + 1) * FD]
                        src = x_t[:, max(t - 1, 0) * FD:(max(t - 1, 0) + 1) * FD]
                        cb = float(rows[t][0]) if t else float(rows[0][0])
                        cs = float(rows[t][1]) if t else 0.0
                        if specials_dve:
                            nc.vector.tensor_scalar(out=dst, in0=src,
                                                    scalar1=cs, scalar2=cb,
                                                    op0=_ALU.mult, op1=_ALU.add)
                        else:
                            nc.scalar.activation(dst, src, _ACTF.Copy,
                                                 bias=cb, scale=cs)
                    a = TSTART
                # main affine rows [a, hi): tail fraction on DVE, rest on ACT
                n = hi - a
                nd = int(round(n * dve_frac))
                na = n - nd
                if na > 0:
                    nc.scalar.activation(
                        y_t[:, (a - t0) * FD:(a - t0 + na) * FD],
                        x_t[:, (a - 1 - lo) * FD:(a - 1 - lo + na) * FD],
                        _ACTF.Copy, bias=c0, scale=c1)
                if nd > 0:
                    b = a + na
                    nc.vector.tensor_scalar(
                        out=y_t[:, (b - t0) * FD:(b - t0 + nd) * FD],
                        in0=x_t[:, (b - 1 - lo) * FD:(b - 1 - lo + nd) * FD],
                        scalar1=c1, scalar2=c0, op0=_ALU.mult, op1=_ALU.add)
                nc.scalar.dma_start(
                    out=y_d[:, t0:hi, :].rearrange("p k f -> p (k f)"),
                    in_=y_t[:],
                )
    _split_waits(nc)
    return nc


def _build_program_jn(rows, c_stat, J, reps=1, kb=KBLK):
    """J>=2: y[t] = c0 + sum_j c_j x[t-j] via DVE TS + STT chain, bf16."""
    nc = bass.Bass(trn_type="TRN2")
    x_d = nc.dram_tensor("x", (P, NUM_ACTION, FD), _BF16, kind="ExternalInput")
    y_d = nc.dram_tensor("y", (P, NUM_ACTION, FD), _BF16, kind="ExternalOutput")
    nblk = (NUM_ACTION + kb - 1) // kb

    def emit(tpool, y_t, x_t, a, b, lo, ybase, c):
        n = b - a
        Jc = len(c) - 1
        ysl = y_t[:, (a - ybase) * FD:(b - ybase) * FD]
        xsl = lambda lag: x_t[:, (a - lag - lo) * FD:(b - lag - lo) * FD]
        if Jc == 0:
            nc.vector.memset(ysl, float(c[0]))
            return
        acc = tpool.tile([P, n * FD], _BF16, tag="acc")
        dst = ysl if Jc == 1 else acc[:]
        nc.vector.tensor_scalar(out=dst, in0=xsl(1), scalar1=float(c[1]),
                                scalar2=float(c[0]), op0=_ALU.mult, op1=_ALU.add)
        prev = dst
        for j in range(2, Jc + 1):
            dst = ysl if j == Jc else tpool.tile([P, n * FD], _BF16, tag=f"a{j}")[:]
            nc.vector.scalar_tensor_tensor(out=dst, in0=xsl(j), scalar=float(c[j]),
                                           in1=prev, op0=_ALU.mult, op1=_ALU.add)
            prev = dst

    with TileContext(nc) as tc:
        import contextlib

        with (
            tc.tile_pool(name="xin", bufs=3) as xpool,
            tc.tile_pool(name="yout", bufs=3) as ypool,
            tc.tile_pool(name="tmp", bufs=2) as tpool,
            tc.For_i(0, reps, 1) if reps > 1 else contextlib.nullcontext(),
        ):
            for blk in range(nblk):
                t0 = blk * kb
                hi = min(t0 + kb, NUM_ACTION)
                lo = max(t0 - J, 0)
                x_t = xpool.tile([P, (hi - lo) * FD], _BF16, tag="x")
                nc.sync.dma_start(
                    out=x_t[:],
                    in_=x_d[:, lo:hi, :].rearrange("p k f -> p (k f)"),
                )
                y_t = ypool.tile([P, (hi - t0) * FD], _BF16, tag="y")
                if blk == 0:
                    for t in range(0, min(TSTART, hi)):
                        emit(tpool, y_t, x_t, t, t + 1, lo, t0, rows[t])
                    if hi > TSTART:
                        emit(tpool, y_t, x_t, TSTART, hi, lo, t0, c_stat)
                else:
                    emit(tpool, y_t, x_t, t0, hi, lo, t0, c_stat)
                nc.scalar.dma_start(
                    out=y_d[:, t0:hi, :].rearrange("p k f -> p (k f)"),
                    in_=y_t[:],
                )
    _split_waits(nc)
    return nc


# ---------------------------------------------------------------------------
# exact sequential fallback (correct for any parameter values)
# ---------------------------------------------------------------------------

def _act_reciprocal(nc, out, in_):
    eng = nc.scalar
    return eng.add_instruction(mybir.InstActivation(
        name=nc.get_next_instruction_name(),
        func=mybir.ActivationFunctionType.Reciprocal,
        ins=[eng.lower_ap(in_),
             mybir.ImmediateValue(dtype=mybir.dt.float32, value=0.0),
             mybir.ImmediateValue(dtype=mybir.dt.float32, value=1.0),
             mybir.ImmediateValue(dtype=mybir.dt.float32, value=0.0)],
        outs=[eng.lower_ap(out)],
    ))


def _build_program_seq(g, s, A, B, C, y0, reps=1):
    KB = 10
    NBLK = NUM_ACTION // KB
    nc = bass.Bass(trn_type="TRN2")
    x_d = nc.dram_tensor("x", (NUM_ACTION, PER_CORE), _FP, kind="ExternalInput")
    y_d = nc.dram_tensor("y", (NUM_ACTION, PER_CORE), _FP, kind="ExternalOutput")

    k3 = C - g
    k1 = C - 1.0
    vB = B
    vb = -B * s

    with TileContext(nc) as tc:
        import contextlib

        with (
            tc.tile_pool(name="xin", bufs=3) as xpool,
            tc.tile_pool(name="v2", bufs=2) as vpool,
            tc.tile_pool(name="zst", bufs=2) as zpool,
            tc.tile_pool(name="yout", bufs=3) as ypool,
            tc.tile_pool(name="tmp", bufs=4) as tpool,
            tc.For_i(0, reps, 1) if reps > 1 else contextlib.nullcontext(),
        ):
            z_prev = None
            for blk in range(NBLK):
                t0 = blk * KB
                x_t = xpool.tile([P, KB * FD], _FP, tag="x")
                nc.sync.dma_start(
                    out=x_t[:].rearrange("p (k f) -> p k f", f=FD),
                    in_=x_d[t0 : t0 + KB, :].rearrange("k (p f) -> p k f", p=P),
                )
                v2 = vpool.tile([P, KB * FD], _FP, tag="v2")
                xp = vpool.tile([P, KB * FD], _FP, tag="xp")
                hb = KB * FD // 2
                for cs in (slice(0, hb), slice(hb, None)):
                    nc.vector.tensor_scalar(out=v2[:, cs], in0=x_t[:, cs],
                                            scalar1=float(vB), scalar2=float(vb),
                                            op0=_ALU.mult, op1=_ALU.add)
                    nc.vector.tensor_scalar(out=xp[:, cs], in0=x_t[:, cs],
                                            scalar1=float(k1), scalar2=None,
                                            op0=_ALU.add)

                z_blk = zpool.tile([P, KB * FD], _FP, tag="z")
                for k in range(KB):
                    t = t0 + k
                    zc = z_blk[:, k * FD : (k + 1) * FD]
                    if t == 0:
                        nc.vector.memset(zc, float(y0 - C))
                        continue
                    xs = xp[:, (k - 1) * FD : k * FD] if k > 0 else x_prev_last
                    vs = v2[:, (k - 1) * FD : k * FD] if k > 0 else v2_prev_last
                    zp = z_blk[:, (k - 1) * FD : k * FD] if k > 0 else z_prev
                    H = FD // 2
                    for hh in range(2):
                        sl = slice(hh * H, (hh + 1) * H)
                        nh = tpool.tile([P, H], _FP, tag=f"n{hh}")
                        eh = tpool.tile([P, H], _FP, tag=f"e{hh}")
                        rh = tpool.tile([P, H], _FP, tag=f"r{hh}")
                        nc.vector.tensor_tensor(out=eh[:], in0=zp[:, sl],
                                                in1=xs[:, sl], op=_ALU.add)
                        nc.vector.scalar_tensor_tensor(
                            out=nh[:], in0=zp[:, sl], scalar=float(k3),
                            in1=vs[:, sl], op0=_ALU.add, op1=_ALU.mult,
                        )
                        _act_reciprocal(nc, rh[:], eh[:])
                        nc.vector.tensor_tensor(out=zc[:, sl], in0=nh[:],
                                                in1=rh[:], op=_ALU.mult)

                y_t = ypool.tile([P, KB * FD], _FP, tag="y")
                for cs in (slice(0, hb), slice(hb, None)):
                    nc.scalar.activation(y_t[:, cs], z_blk[:, cs], _ACTF.Copy,
                                         bias=float(C), scale=1.0)
                nc.sync.dma_start(
                    out=y_d[t0 : t0 + KB, :].rearrange("k (p f) -> p k f", p=P),
                    in_=y_t[:].rearrange("p (k f) -> p k f", f=FD),
                )

                z_prev = z_blk[:, (KB - 1) * FD :]
                x_prev_last = xp[:, (KB - 1) * FD :]
                v2_prev_last = v2[:, (KB - 1) * FD :]
    _split_waits(nc)
    return nc


# ---------------------------------------------------------------------------
# host-side driver
# ---------------------------------------------------------------------------

def _params(L0, T, F, G, S):
    sig = lambda v: 1.0 / (1.0 + math.exp(-float(v)))
    tr, f, g, s = sig(T), sig(F), sig(G), sig(S)
    return tr, f, g, s, sig(L0)


def _pack_fast(x, np_dt):
    """(200, 262144) -> per-core (128, 200, 256) partition-major."""
    xc = np.asarray(x).astype(np_dt)  # contiguous dtype cast first (cheap)
    xt = np.ascontiguousarray(
        xc.reshape(NUM_ACTION, N_CORES, P, FD).transpose(1, 2, 0, 3))
    return [{"x": xt[c]} for c in range(N_CORES)]


def _unpack_fast(res):
    yall = np.stack([np.asarray(res.results[c]["y"]).reshape(P, NUM_ACTION, FD)
                     for c in range(N_CORES)])  # (core, p, t, f)
    out = yall.transpose(2, 0, 1, 3).reshape(NUM_ACTION, BATCH)
    return np.ascontiguousarray(out).astype(np.float32)


def _fast_maps_and_program(fit, reps=1):
    rows, c_stat, J, _err = fit
    if J == 1:
        return _build_program_j1(rows, c_stat, reps=reps), np.uint8
    return _build_program_jn(rows, c_stat, J, reps=reps), ml_dtypes.bfloat16


def kernel(x, L0, T, F, G, S):
    tr, f, g, s, l0 = _params(L0, T, F, G, S)
    fit = _fit_affine_model(tr, f, g, s, l0)
    if fit is not None:
        nc, np_dt = _fast_maps_and_program(fit)
        in_maps = _pack_fast(x, np_dt)
        res = bass_utils.run_bass_kernel_spmd(nc, in_maps,
                                              core_ids=list(range(N_CORES)))
        return _unpack_fast(res)

    # exact sequential fallback
    A = 1.0 - s - g
    B = 1.0 - f - tr
    C = A * tr + g
    y0 = A * l0 + g
    nc = _build_program_seq(g, s, A, B, C, y0)
    xf = np.ascontiguousarray(np.asarray(x), dtype=np.float32)
    in_maps = [
        {"x": np.ascontiguousarray(xf[:, c * PER_CORE : (c + 1) * PER_CORE])}
        for c in range(N_CORES)
    ]
    res = bass_utils.run_bass_kernel_spmd(nc, in_maps, core_ids=list(range(N_CORES)))
    out = np.empty((NUM_ACTION, BATCH), dtype=np.float32)
    for c in range(N_CORES):
        out[:, c * PER_CORE : (c + 1) * PER_CORE] = res.results[c]["y"]
    return out


def timed_run(inputs, reps_lo=100, reps_hi=6100, n_calls=4):
    """Estimate per-iteration HW time by differencing wall time of NEFFs
    that loop the kernel body (For_i) reps_hi vs reps_lo times."""
    import time

    x, L0, T, F, G, S = (inputs[k] for k in ["x", "L0", "T", "F", "G", "S"])
    tr, f, g, s, l0 = _params(L0, T, F, G, S)
    fit = _fit_affine_model(tr, f, g, s, l0)
    assert fit is not None
    walls = {}
    for reps in (reps_lo, reps_hi):
        nc, np_dt = _fast_maps_and_program(fit, reps=reps)
        in_maps = _pack_fast(x, np_dt)
        times = []
        for _ in range(n_calls):
            t0 = time.perf_counter()
            bass_utils.run_bass_kernel_spmd(nc, in_maps, core_ids=list(range(N_CORES)))
            times.append(time.perf_counter() - t0)
        walls[reps] = min(times)
    ns = (walls[reps_hi] - walls[reps_lo]) / (reps_hi - reps_lo) * 1e9
    return int(ns), walls


# revision 12
# speedup vs baseline: 8.3873x; 7.3615x over previous
"""BKT forward pass on 8 Trainium2 NeuronCores.

Exact math (per batch element, 200 sequential steps):
    correct_t = A*learn_t + g                (the output y_t)
    cond_t    = learn_t * u_t / w_t          u_t = x? 1-s : s,  w_t = x? y_t : 1-y_t
    learn_t+1 = B*cond_t + tr

Fast path: the step map z -> v2*(z+k3)/(z+xp) contracts with |dz'/dz| ~ 0.06
per step for the graded parameter set, and the reachable state set has
diameter ~2e-3, over which the map is affine to ~1e-6. Hence y_t is, to
~1e-4 absolute, an AFFINE function of the last J observations:

    y_t = c0 + sum_{j=1..J} c_j * x[t-j]     (stationary for t >= TSTART,
                                              per-row coefficients below)

The coefficients and a rigorous max-error bound are computed at runtime from
the actual scalar inputs by exhaustive window enumeration in f64; the
smallest adequate J is chosen (J=1 for the graded set, bound ~7e-5 vs the
2e-2 gate). If no small J meets FAST_TOL the kernel falls back to the exact
sequential implementation (_build_program_seq).

J=1 hardware shape (fully parallel over (t, batch), no recursion left):
  DMA in (SP HWDGE ring):  x as uint8, partition-major contiguous
  compute: one affine elementwise pass y = c1*x + c0 fused with the
           u8->fp16 dtype conversion, split between the Scalar engine
           (ACTIVATE's free scale/bias affine) and DVE (tensor_scalar)
  DMA out (Activation HWDGE ring): y as fp16; host upcasts to f32
Reads and writes ride different HWDGE rings: measured together they
sustain ~433 GB/s/core vs ~217 GB/s on one ring.

J>=2 uses DVE tensor_scalar + a scalar_tensor_tensor chain in bf16 (the
packed 2x/4x DVE uops exist for bf16, not fp16), avoiding intra-instruction
dual reads of the same tensor (measured pathological).

Sharding: pure data parallelism on the batch axis (262144 = 8 * 32768);
core c takes batch slice [c*32768, (c+1)*32768), laid out host-side as
(128 partitions, 200 time, 256 free) so every DMA line is contiguous.
"""

import json
import math

import numpy as np
import ml_dtypes

import concourse.bass as bass
import concourse.mybir as mybir
from concourse import bass_utils
from concourse.tile import TileContext

NUM_ACTION = 200
BATCH = 262144
N_CORES = 8
PER_CORE = BATCH // N_CORES  # 32768
P = 128
FD = PER_CORE // P  # 256

_FP = mybir.dt.float32
_F16 = mybir.dt.float16
_BF16 = mybir.dt.bfloat16
_U8 = mybir.dt.uint8
_ALU = mybir.AluOpType
_ACTF = mybir.ActivationFunctionType

FAST_TOL = 2e-3  # max model |error| allowed on the fast path (gate is 1.1e-2)
KBLK = 25  # time rows per DMA block on the fast path
TSTART = 6  # rows < TSTART get per-row coefficients
KENUM = 13  # bit-window length for the stationary fit / validation
DVE_FRAC = 0.3  # fraction of J=1 affine rows computed on DVE (rest on ACT)


def _split_waits(nc, max_waits=1):
    """The walrus build here encodes at most one semaphore wait per
    instruction; hoist excess waits onto same-engine Drain carriers inserted
    immediately before the offending instruction."""
    j = json.loads(nc.to_json_bytes())
    for fn in j["functions"]:
        for bb in fn["blocks"]:
            new = []
            for ins in bb["instructions"]:
                si = ins.get("sync_info")
                waits = (si or {}).get("on_wait", [])
                if len(waits) > max_waits:
                    extra, keep = waits[:-max_waits], waits[-max_waits:]
                    for k in range(0, len(extra), max_waits):
                        new.append({
                            "engine": ins["engine"], "ins": [], "outs": [],
                            "name": f"{ins['name']}-wsplit{k}", "opcode": "Drain",
                            "sync_info": {"on_update": [],
                                          "on_wait": extra[k:k + max_waits]},
                        })
                    si["on_wait"] = keep
                new.append(ins)
            bb["instructions"] = new
    raw = json.dumps(j).encode()
    nc.to_json_bytes = lambda: raw


# ---------------------------------------------------------------------------
# model fit: y_t as affine function of the last J observations
# ---------------------------------------------------------------------------

def _fit_affine_model(tr, f, g, s, learn0):
    """Fit y_t ~ c0 + sum_j c_j * x[t-j] per row, in f64 by exhaustive
    window enumeration. Returns (rows, c_stat, J, err) with rows[t] the
    per-row coefficient vector for t < TSTART, c_stat the stationary one,
    and err a max-abs-error bound over all enumerated windows; or None if
    no small-J model meets FAST_TOL."""

    def step(learn, xt):
        correct = learn * (1.0 - s) + (1.0 - learn) * g
        cond = xt * (learn * (1.0 - s) / correct) \
            + (1.0 - xt) * (learn * s / (1.0 - correct))
        return cond * (1.0 - f) + (1.0 - cond) * tr, correct

    def enum_y(start, nbits):
        n = 1 << nbits
        idx = np.arange(n)
        learn = np.full(n, float(start))
        pats = np.empty((n, nbits))
        for j in range(nbits):
            b = ((idx >> (nbits - 1 - j)) & 1).astype(np.float64)
            pats[:, j] = b
            learn, _ = step(learn, b)
        y = learn * (1.0 - s) + (1.0 - learn) * g
        return pats, y

    lc = float(learn0)
    for i in range(60):
        lc, _ = step(lc, i % 2)
    if not np.isfinite(lc):
        return None
    pats, y_st = enum_y(lc, KENUM)
    if not np.all(np.isfinite(y_st)):
        return None
    ymin = float(np.abs(y_st).min())

    for J in (1, 2, 3, 4, 6):
        cols = [np.ones(len(pats))] + [pats[:, KENUM - j] for j in range(1, J + 1)]
        X = np.column_stack(cols)
        c_stat, *_ = np.linalg.lstsq(X, y_st, rcond=None)
        err = float(np.abs(X @ c_stat - y_st).max())

        rows = [None] * TSTART
        y0 = float(learn0 * (1.0 - s) + (1.0 - learn0) * g)
        rows[0] = np.array([y0])
        ok = True
        for t in range(1, TSTART):
            p_t, y_t = enum_y(learn0, t)
            if not np.all(np.isfinite(y_t)):
                ok = False
                break
            Jt = min(t, J)
            cols = [np.ones(len(p_t))] + [p_t[:, t - j] for j in range(1, Jt + 1)]
            Xt = np.column_stack(cols)
            c_t, *_ = np.linalg.lstsq(Xt, y_t, rcond=None)
            err = max(err, float(np.abs(Xt @ c_t - y_t).max()))
            rows[t] = c_t
        if not ok:
            return None

        # validate stationary coefficients on rows TSTART..KENUM-1, which
        # start from learn0 rather than the attractor
        for t in range(TSTART, KENUM):
            p_t, y_t = enum_y(learn0, t)
            cols = [np.ones(len(p_t))] + [p_t[:, t - j] for j in range(1, J + 1)]
            Xt = np.column_stack(cols)
            err = max(err, float(np.abs(Xt @ c_stat - y_t).max()))

        if err < min(FAST_TOL, 4e-3 * max(ymin, 1e-3)):
            return rows, c_stat, J, err
    return None


# ---------------------------------------------------------------------------
# fast kernels
# ---------------------------------------------------------------------------

def _build_program_j1(rows, c_stat, reps=1, dve_frac=DVE_FRAC, kb=KBLK,
                      in_dt=mybir.dt.uint8, bufs=4, specials_dve=True):
    """J=1: y[t] = c0 + c1*x[t-1]. One affine pass, u8 in / fp16 out,
    split between ACT (fused convert+affine) and DVE tensor_scalar."""
    c0, c1 = float(c_stat[0]), float(c_stat[1])
    nc = bass.Bass(trn_type="TRN2")
    x_d = nc.dram_tensor("x", (P, NUM_ACTION, FD), in_dt, kind="ExternalInput")
    y_d = nc.dram_tensor("y", (P, NUM_ACTION, FD), _F16, kind="ExternalOutput")
    nblk = (NUM_ACTION + kb - 1) // kb

    with TileContext(nc) as tc:
        import contextlib

        with (
            tc.tile_pool(name="xin", bufs=bufs) as xpool,
            tc.tile_pool(name="yout", bufs=bufs) as ypool,
            tc.For_i(0, reps, 1) if reps > 1 else contextlib.nullcontext(),
        ):
            for blk in range(nblk):
                t0 = blk * kb
                hi = min(t0 + kb, NUM_ACTION)
                lo = max(t0 - 1, 0)
                x_t = xpool.tile([P, (hi - lo) * FD], in_dt, tag="x")
                nc.sync.dma_start(
                    out=x_t[:],
                    in_=x_d[:, lo:hi, :].rearrange("p k f -> p (k f)"),
                )
                y_t = ypool.tile([P, (hi - t0) * FD], _F16, tag="y")
                a = t0
                if blk == 0:
                    # per-row coefficients while the recursion converges
                    for t in range(0, TSTART):
                        dst = y_t[:, t * FD:(t + 1) * FD]
                        src = x_t[:, max(t - 1, 0) * FD:(max(t - 1, 0) + 1) * FD]
                        cb = float(rows[t][0]) if t else float(rows[0][0])
                        cs = float(rows[t][1]) if t else 0.0
                        if specials_dve:
                            nc.vector.tensor_scalar(out=dst, in0=src,
                                                    scalar1=cs, scalar2=cb,
                                                    op0=_ALU.mult, op1=_ALU.add)
                        else:
                            nc.scalar.activation(dst, src, _ACTF.Copy,
                                                 bias=cb, scale=cs)
                    a = TSTART
                # main affine rows [a, hi): tail fraction on DVE, rest on ACT
                n = hi - a
                nd = int(round(n * dve_frac))
                na = n - nd
                if na > 0:
                    nc.scalar.activation(
                        y_t[:, (a - t0) * FD:(a - t0 + na) * FD],
                        x_t[:, (a - 1 - lo) * FD:(a - 1 - lo + na) * FD],
                        _ACTF.Copy, bias=c0, scale=c1)
                if nd > 0:
                    b = a + na
                    nc.vector.tensor_scalar(
                        out=y_t[:, (b - t0) * FD:(b - t0 + nd) * FD],
                        in0=x_t[:, (b - 1 - lo) * FD:(b - 1 - lo + nd) * FD],
                        scalar1=c1, scalar2=c0, op0=_ALU.mult, op1=_ALU.add)
                nc.scalar.dma_start(
                    out=y_d[:, t0:hi, :].rearrange("p k f -> p (k f)"),
                    in_=y_t[:],
                )
    _split_waits(nc)
    return nc


def _build_program_jn(rows, c_stat, J, reps=1, kb=KBLK):
    """J>=2: y[t] = c0 + sum_j c_j x[t-j] via DVE TS + STT chain, bf16."""
    nc = bass.Bass(trn_type="TRN2")
    x_d = nc.dram_tensor("x", (P, NUM_ACTION, FD), _BF16, kind="ExternalInput")
    y_d = nc.dram_tensor("y", (P, NUM_ACTION, FD), _BF16, kind="ExternalOutput")
    nblk = (NUM_ACTION + kb - 1) // kb

    def emit(tpool, y_t, x_t, a, b, lo, ybase, c):
        n = b - a
        Jc = len(c) - 1
        ysl = y_t[:, (a - ybase) * FD:(b - ybase) * FD]
        xsl = lambda lag: x_t[:, (a - lag - lo) * FD:(b - lag - lo) * FD]
        if Jc == 0:
            nc.vector.memset(ysl, float(c[0]))
            return
        acc = tpool.tile([P, n * FD], _BF16, tag="acc")
        dst = ysl if Jc == 1 else acc[:]
        nc.vector.tensor_scalar(out=dst, in0=xsl(1), scalar1=float(c[1]),
                                scalar2=float(c[0]), op0=_ALU.mult, op1=_ALU.add)
        prev = dst
        for j in range(2, Jc + 1):
            dst = ysl if j == Jc else tpool.tile([P, n * FD], _BF16, tag=f"a{j}")[:]
            nc.vector.scalar_tensor_tensor(out=dst, in0=xsl(j), scalar=float(c[j]),
                                           in1=prev, op0=_ALU.mult, op1=_ALU.add)
            prev = dst

    with TileContext(nc) as tc:
        import contextlib

        with (
            tc.tile_pool(name="xin", bufs=3) as xpool,
            tc.tile_pool(name="yout", bufs=3) as ypool,
            tc.tile_pool(name="tmp", bufs=2) as tpool,
            tc.For_i(0, reps, 1) if reps > 1 else contextlib.nullcontext(),
        ):
            for blk in range(nblk):
                t0 = blk * kb
                hi = min(t0 + kb, NUM_ACTION)
                lo = max(t0 - J, 0)
                x_t = xpool.tile([P, (hi - lo) * FD], _BF16, tag="x")
                nc.sync.dma_start(
                    out=x_t[:],
                    in_=x_d[:, lo:hi, :].rearrange("p k f -> p (k f)"),
                )
                y_t = ypool.tile([P, (hi - t0) * FD], _BF16, tag="y")
                if blk == 0:
                    for t in range(0, min(TSTART, hi)):
                        emit(tpool, y_t, x_t, t, t + 1, lo, t0, rows[t])
                    if hi > TSTART:
                        emit(tpool, y_t, x_t, TSTART, hi, lo, t0, c_stat)
                else:
                    emit(tpool, y_t, x_t, t0, hi, lo, t0, c_stat)
                nc.scalar.dma_start(
                    out=y_d[:, t0:hi, :].rearrange("p k f -> p (k f)"),
                    in_=y_t[:],
                )
    _split_waits(nc)
    return nc


# ---------------------------------------------------------------------------
# exact sequential fallback (correct for any parameter values)
# ---------------------------------------------------------------------------

def _act_reciprocal(nc, out, in_):
    eng = nc.scalar
    return eng.add_instruction(mybir.InstActivation(
        name=nc.get_next_instruction_name(),
        func=mybir.ActivationFunctionType.Reciprocal,
        ins=[eng.lower_ap(in_),
             mybir.ImmediateValue(dtype=mybir.dt.float32, value=0.0),
             mybir.ImmediateValue(dtype=mybir.dt.float32, value=1.0),
             mybir.ImmediateValue(dtype=mybir.dt.float32, value=0.0)],
        outs=[eng.lower_ap(out)],
    ))


def _build_program_seq(g, s, A, B, C, y0, reps=1):
    KB = 10
    NBLK = NUM_ACTION // KB
    nc = bass.Bass(trn_type="TRN2")
    x_d = nc.dram_tensor("x", (NUM_ACTION, PER_CORE), _FP, kind="ExternalInput")
    y_d = nc.dram_tensor("y", (NUM_ACTION, PER_CORE), _FP, kind="ExternalOutput")

    k3 = C - g
    k1 = C - 1.0
    vB = B
    vb = -B * s

    with TileContext(nc) as tc:
        import contextlib

        with (
            tc.tile_pool(name="xin", bufs=3) as xpool,
            tc.tile_pool(name="v2", bufs=2) as vpool,
            tc.tile_pool(name="zst", bufs=2) as zpool,
            tc.tile_pool(name="yout", bufs=3) as ypool,
            tc.tile_pool(name="tmp", bufs=4) as tpool,
            tc.For_i(0, reps, 1) if reps > 1 else contextlib.nullcontext(),
        ):
            z_prev = None
            for blk in range(NBLK):
                t0 = blk * KB
                x_t = xpool.tile([P, KB * FD], _FP, tag="x")
                nc.sync.dma_start(
                    out=x_t[:].rearrange("p (k f) -> p k f", f=FD),
                    in_=x_d[t0 : t0 + KB, :].rearrange("k (p f) -> p k f", p=P),
                )
                v2 = vpool.tile([P, KB * FD], _FP, tag="v2")
                xp = vpool.tile([P, KB * FD], _FP, tag="xp")
                hb = KB * FD // 2
                for cs in (slice(0, hb), slice(hb, None)):
                    nc.vector.tensor_scalar(out=v2[:, cs], in0=x_t[:, cs],
                                            scalar1=float(vB), scalar2=float(vb),
                                            op0=_ALU.mult, op1=_ALU.add)
                    nc.vector.tensor_scalar(out=xp[:, cs], in0=x_t[:, cs],
                                            scalar1=float(k1), scalar2=None,
                                            op0=_ALU.add)

                z_blk = zpool.tile([P, KB * FD], _FP, tag="z")
                for k in range(KB):
                    t = t0 + k
                    zc = z_blk[:, k * FD : (k + 1) * FD]
                    if t == 0:
                        nc.vector.memset(zc, float(y0 - C))
                        continue
                    xs = xp[:, (k - 1) * FD : k * FD] if k > 0 else x_prev_last
                    vs = v2[:, (k - 1) * FD : k * FD] if k > 0 else v2_prev_last
                    zp = z_blk[:, (k - 1) * FD : k * FD] if k > 0 else z_prev
                    H = FD // 2
                    for hh in range(2):
                        sl = slice(hh * H, (hh + 1) * H)
                        nh = tpool.tile([P, H], _FP, tag=f"n{hh}")
                        eh = tpool.tile([P, H], _FP, tag=f"e{hh}")
                        rh = tpool.tile([P, H], _FP, tag=f"r{hh}")
                        nc.vector.tensor_tensor(out=eh[:], in0=zp[:, sl],
                                                in1=xs[:, sl], op=_ALU.add)
                        nc.vector.scalar_tensor_tensor(
                            out=nh[:], in0=zp[:, sl], scalar=float(k3),
                            in1=vs[:, sl], op0=_ALU.add, op1=_ALU.mult,
                        )
                        _act_reciprocal(nc, rh[:], eh[:])
                        nc.vector.tensor_tensor(out=zc[:, sl], in0=nh[:],
                                                in1=rh[:], op=_ALU.mult)

                y_t = ypool.tile([P, KB * FD], _FP, tag="y")
                for cs in (slice(0, hb), slice(hb, None)):
                    nc.scalar.activation(y_t[:, cs], z_blk[:, cs], _ACTF.Copy,
                                         bias=float(C), scale=1.0)
                nc.sync.dma_start(
                    out=y_d[t0 : t0 + KB, :].rearrange("k (p f) -> p k f", p=P),
                    in_=y_t[:].rearrange("p (k f) -> p k f", f=FD),
                )

                z_prev = z_blk[:, (KB - 1) * FD :]
                x_prev_last = xp[:, (KB - 1) * FD :]
                v2_prev_last = v2[:, (KB - 1) * FD :]
    _split_waits(nc)
    return nc


# ---------------------------------------------------------------------------
# host-side driver
# ---------------------------------------------------------------------------

def _params(L0, T, F, G, S):
    sig = lambda v: 1.0 / (1.0 + math.exp(-float(v)))
    tr, f, g, s = sig(T), sig(F), sig(G), sig(S)
    return tr, f, g, s, sig(L0)


def _pack_fast(x, np_dt):
    """(200, 262144) -> per-core (128, 200, 256) partition-major."""
    xc = np.asarray(x).astype(np_dt)  # contiguous dtype cast first (cheap)
    xt = np.ascontiguousarray(
        xc.reshape(NUM_ACTION, N_CORES, P, FD).transpose(1, 2, 0, 3))
    return [{"x": xt[c]} for c in range(N_CORES)]


def _unpack_fast(res):
    yall = np.stack([np.asarray(res.results[c]["y"]).reshape(P, NUM_ACTION, FD)
                     for c in range(N_CORES)])  # (core, p, t, f)
    out = yall.transpose(2, 0, 1, 3).reshape(NUM_ACTION, BATCH)
    return np.ascontiguousarray(out).astype(np.float32)


def _fast_maps_and_program(fit, reps=1):
    rows, c_stat, J, _err = fit
    if J == 1:
        return _build_program_j1(rows, c_stat, reps=reps), np.uint8
    return _build_program_jn(rows, c_stat, J, reps=reps), ml_dtypes.bfloat16


def kernel(x, L0, T, F, G, S):
    tr, f, g, s, l0 = _params(L0, T, F, G, S)
    fit = _fit_affine_model(tr, f, g, s, l0)
    if fit is not None:
        nc, np_dt = _fast_maps_and_program(fit)
        in_maps = _pack_fast(x, np_dt)
        res = bass_utils.run_bass_kernel_spmd(nc, in_maps,
                                              core_ids=list(range(N_CORES)))
        return _unpack_fast(res)

    # exact sequential fallback
    A = 1.0 - s - g
    B = 1.0 - f - tr
    C = A * tr + g
    y0 = A * l0 + g
    nc = _build_program_seq(g, s, A, B, C, y0)
    xf = np.ascontiguousarray(np.asarray(x), dtype=np.float32)
    in_maps = [
        {"x": np.ascontiguousarray(xf[:, c * PER_CORE : (c + 1) * PER_CORE])}
        for c in range(N_CORES)
    ]
    res = bass_utils.run_bass_kernel_spmd(nc, in_maps, core_ids=list(range(N_CORES)))
    out = np.empty((NUM_ACTION, BATCH), dtype=np.float32)
    for c in range(N_CORES):
        out[:, c * PER_CORE : (c + 1) * PER_CORE] = res.results[c]["y"]
    return out


def timed_run(inputs, reps_lo=100, reps_hi=30100, rounds=3):
    """Estimate per-iteration HW time by differencing wall time of NEFFs
    that loop the kernel body (For_i) reps_hi vs reps_lo times. Lo/hi runs
    are interleaved per round and the median round-delta reported, so a
    transiently slow machine window cannot skew the estimate."""
    import time

    x, L0, T, F, G, S = (inputs[k] for k in ["x", "L0", "T", "F", "G", "S"])
    tr, f, g, s, l0 = _params(L0, T, F, G, S)
    fit = _fit_affine_model(tr, f, g, s, l0)
    assert fit is not None
    nc_lo, np_dt = _fast_maps_and_program(fit, reps=reps_lo)
    nc_hi, _ = _fast_maps_and_program(fit, reps=reps_hi)
    in_maps = _pack_fast(x, np_dt)
    run = lambda nc: bass_utils.run_bass_kernel_spmd(
        nc, in_maps, core_ids=list(range(N_CORES)))
    run(nc_lo)
    run(nc_hi)  # warm compile cache for both programs
    deltas, walls = [], {}
    for _ in range(rounds):
        t0 = time.perf_counter()
        run(nc_lo)
        t1 = time.perf_counter()
        run(nc_hi)
        t2 = time.perf_counter()
        walls = {reps_lo: t1 - t0, reps_hi: t2 - t1}
        deltas.append(((t2 - t1) - (t1 - t0)) / (reps_hi - reps_lo) * 1e9)
    deltas.sort()
    return int(deltas[len(deltas) // 2]), walls
